# revision 1
# baseline (speedup 1.0000x reference)
"""Trainium2 Bass kernel for a custom transformer encoder layer
(pre-LN, RoPE-on-full-d_model attention, SwiGLU FFN).

Sharding: DP over batch (2 groups of 4 cores) x TP within group:
  - attention: 4 heads/core (head pairs {h, h+8} so RoPE stays local),
  - FFN: dim_feedforward/4 per core,
  - residuals folded into the two grouped AllReduces (src/4 added on every
    core pre-AR so the CCE sum reconstructs src exactly).

Dataflow per core (batch b, core j in group):
  LN1 (token-major, stats on src/4 with eps/16 -> scale-invariant)
  -> PE-transpose to feature-major bf16 x_hat
  -> q,k feature-major + fused RoPE; v token-major (+ones col for softmax sums)
  -> per 512-token chunk: scores^T = k^T q per head -> exp -> av matmul with
     ones-row giving softmax sums -> reciprocal+K=1-broadcast-matmul normalize
  -> out-proj (token-major) + src/4 -> AllReduce(group) = src2
  -> LN2 token-major -> transpose -> SwiGLU FFN (ff-slice) -> down-proj
     (token-major) + src2/4 -> AllReduce(group) = final output chunk.
"""
import sys

sys.path.insert(0, "/opt/trn_rl_repo")

import numpy as np
import ml_dtypes

import concourse.bass as bass
import concourse.mybir as mybir
from concourse import masks, tile
from concourse.bass_utils import run_bass_kernel_spmd

BF = ml_dtypes.bfloat16
F32 = mybir.dt.float32
BF16 = mybir.dt.bfloat16
AF = mybir.ActivationFunctionType
ALU = mybir.AluOpType

B, D, H, HD, FF = 2, 1024, 16, 64, 4096
EPS = 1e-5
N_CORES = 8

# ---------------------------------------------------------------------------
# Workaround: this neuronxcc build rejects >1 sem wait on the CTRL-queue (SP)
# drain Tile emits at context exit. Split extra waits onto chained SP nops
# (same FIFO queue -> semantics preserved).
_MAXW = 1


def _split_waits(nc, inst):
    si = inst.sync_info
    if si is None:
        return
    waits = list(si.on_wait)
    if len(waits) <= _MAXW:
        return
    inst.sync_info = mybir.SyncInfo(on_wait=waits[:_MAXW], on_update=list(si.on_update))
    for i in range(_MAXW, len(waits), _MAXW):
        ni = nc.sync.nop(nofuse=True)
        ni.ins.sync_info = mybir.SyncInfo(on_wait=waits[i : i + _MAXW], on_update=[])


_SPLIT_N = [0]


def split_all_waits(nc, maxw=1):
    """Post-pass: this walrus build rejects instructions carrying more than a
    couple of sem waits. Move extra waits onto same-engine nops inserted
    immediately before the offending instruction (per-engine FIFO order
    makes this equivalent)."""
    for f in nc.m.functions:
        for bb in f.blocks:
            out = []
            changed = False
            for inst in bb.instructions:
                si = getattr(inst, "sync_info", None)
                waits = list(si.on_wait) if si is not None else []
                if len(waits) > maxw:
                    for i in range(maxw, len(waits), maxw):
                        _SPLIT_N[0] += 1
                        nop = mybir.InstNoOp(
                            name=f"I-wsplit-{_SPLIT_N[0]}", engine=inst.engine,
                            ins=[], outs=[],
                        )
                        nop.sync_info = mybir.SyncInfo(
                            on_wait=waits[i:i + maxw], on_update=[]
                        )
                        out.append(nop)
                        changed = True
                    inst.sync_info = mybir.SyncInfo(
                        on_wait=waits[:maxw], on_update=list(si.on_update)
                    )
                out.append(inst)
            if changed:
                bb.instructions = out


def _patched_drain_and_barrier(self, tick_clock, wait_clock):
    nc = self.nc
    drain_inst = nc.sync.drain()
    wait_clock.add_sem_waits(
        drain_inst.ins, tile.ScopedClock({None: tick_clock.global_clock})
    )
    _split_waits(nc, drain_inst.ins)
    nc.all_engine_barrier()
    assert self.sems is not None
    popped = nc._tile_sem_poison_stack.pop()
    assert popped is self._sem_poison
    nc.clear_and_free_semaphores(list(self.sems.allocated().values()))
    nc.all_engine_barrier()


tile.TileContext._drain_and_barrier = _patched_drain_and_barrier
# ---------------------------------------------------------------------------


def build_bass(S=2048, CT=512, use_silu=True, wait_split=True):
    """Emit the SPMD program. CT = tokens per pipeline chunk."""
    NT = S // 128          # 128-token tiles
    NCH = S // CT          # chunks
    CM = CT // 128         # 128-token tiles per chunk
    NQ = CT // 512 if CT >= 512 else 1  # q-tile width for qk projections
    QW = min(S, 512)       # matmul N for q/k projection over full seq
    NQK = S // QW

    nc = bass.Bass(target_bir_lowering=False, debug=False)

    # --- I/O ---
    xsrc4 = nc.dram_tensor("xsrc4", [S, D], F32, kind="ExternalInput")
    cosA = nc.dram_tensor("cosA", [128, S], BF16, kind="ExternalInput")
    sinA = nc.dram_tensor("sinA", [128, S], BF16, kind="ExternalInput")
    cosB = nc.dram_tensor("cosB", [128, S], BF16, kind="ExternalInput")
    sinB = nc.dram_tensor("sinB", [128, S], BF16, kind="ExternalInput")
    wq_d = nc.dram_tensor("wq", [128, 8, 256], BF16, kind="ExternalInput")
    wk_d = nc.dram_tensor("wk", [128, 8, 256], BF16, kind="ExternalInput")
    wv_d = nc.dram_tensor("wv", [128, 8, 256], BF16, kind="ExternalInput")
    wo_d = nc.dram_tensor("wo", [128, 2, D], BF16, kind="ExternalInput")
    w1_d = nc.dram_tensor("w1t", [128, 8, FF // 4], BF16, kind="ExternalInput")
    w3_d = nc.dram_tensor("w3t", [128, 8, FF // 4], BF16, kind="ExternalInput")
    w2_d = nc.dram_tensor("w2t", [128, 8, D], BF16, kind="ExternalInput")
    y = nc.dram_tensor("y", [S, D], F32, kind="ExternalOutput")

    # --- internal DRAM for collectives ---
    ar1_in = nc.dram_tensor("ar1_in", [S, D], F32)
    ar1_out = nc.dram_tensor("ar1_out", [S, D], F32)
    ar2_in = nc.dram_tensor("ar2_in", [S, D], F32)
    ar2_out = nc.dram_tensor("ar2_out", [S, D], F32)
    groups = [[0, 1, 2, 3], [4, 5, 6, 7]]

    FFS = FF // 4          # ff slice per core
    NFF = FFS // 128

    with tile.TileContext(nc) as tc:
        with (
            tc.tile_pool(name="consts", bufs=1) as cpool,
            tc.tile_pool(name="weights", bufs=1) as wpool,
            tc.tile_pool(name="persist", bufs=1) as ppool,
            tc.tile_pool(name="psum", bufs=1, space="PSUM") as psum,
            tc.tile_pool(name="work", bufs=2) as work,
            tc.tile_pool(name="stream", bufs=3) as stream,
        ):
            # consts
            ident = cpool.tile([128, 128], BF16)
            masks.make_identity(nc, ident[:])
            ones64 = cpool.tile([1, 64], F32)
            nc.vector.memset(ones64[:], 1.0)

            # weights -> SBUF
            wq = wpool.tile([128, 8, 256], BF16)
            wk = wpool.tile([128, 8, 256], BF16)
            wv = wpool.tile([128, 8, 256], BF16)
            wo = wpool.tile([128, 2, D], BF16)
            w1 = wpool.tile([128, 8, FFS], BF16)
            w3 = wpool.tile([128, 8, FFS], BF16)
            w2 = wpool.tile([128, 8, D], BF16)
            for sb, dr in ((wq, wq_d), (wk, wk_d), (wv, wv_d), (wo, wo_d),
                           (w1, w1_d), (w3, w3_d), (w2, w2_d)):
                nc.sync.dma_start(sb[:], dr[:])

            # persistent activations
            q_t = [ppool.tile([128, S], BF16, tag=f"q{g}", name=f"q{g}")
                   for g in range(2)]
            k_t = [ppool.tile([128, S], BF16, tag=f"k{g}", name=f"k{g}")
                   for g in range(2)]
            v_sb = ppool.tile([128, NT, 4, 65], BF16)
            nc.vector.memset(v_sb[:, :, :, 64:65], 1.0)

            with (
                tc.tile_pool(name="ab", bufs=1) as ab,
            ):
                xhat = ab.tile([128, 8, S], BF16)
                cs = {}
                for nm, dr in (("cosA", cosA), ("sinA", sinA),
                               ("cosB", cosB), ("sinB", sinB)):
                    t = ab.tile([128, S], BF16, tag=nm)
                    nc.sync.dma_start(t[:], dr[:])
                    cs[nm] = t

                # ---- Phase A: LN1 + transpose ----
                for ti in range(NT):
                    sl = slice(ti * 128, ti * 128 + 128)
                    src_t = stream.tile([128, D], F32, tag="stream", bufs=4)
                    nc.sync.dma_start(src_t[:], xsrc4[sl, :])
                    st = work.tile([128, 2, 6], F32, tag="st")
                    nc.vector.bn_stats(st[:, 0, :], src_t[:, 0:512])
                    nc.vector.bn_stats(st[:, 1, :], src_t[:, 512:1024])
                    mv = work.tile([128, 2], F32, tag="mv")
                    nc.vector.bn_aggr(mv[:], st[:])
                    vareps = work.tile([128, 1], F32, tag="ve")
                    nc.vector.tensor_scalar_add(vareps[:], mv[:, 1:2], EPS / 16.0)
                    stdv = work.tile([128, 1], F32, tag="sd")
                    nc.scalar.activation(stdv[:], vareps[:], AF.Sqrt)
                    rstd = work.tile([128, 1], F32, tag="rs")
                    nc.vector.reciprocal(rstd[:], stdv[:])
                    xn = work.tile([128, D], BF16, tag="xn")
                    nc.vector.tensor_scalar(
                        xn[:], src_t[:], mv[:, 0:1], rstd[:],
                        ALU.subtract, ALU.mult,
                    )
                    for half in range(2):
                        tp = psum.tile([128, 4, 128], BF16, tag="tp", bufs=2)
                        for c in range(4):
                            nc.tensor.transpose(
                                tp[:, c, :],
                                xn[:, (half * 4 + c) * 128:(half * 4 + c + 1) * 128],
                                ident[:],
                            )
                        nc.any.tensor_copy(xhat[:, half * 4:half * 4 + 4, sl], tp[:])

                # ---- Phase B: q, k (+RoPE) and v ----
                for which, wmat, outAB in (("k", wk, k_t), ("q", wq, q_t)):
                    for ntl in range(NQK):
                        nsl = slice(ntl * QW, (ntl + 1) * QW)
                        pA = psum.tile([128, QW], F32, tag="acc", bufs=3)
                        for kt in range(8):
                            nc.tensor.matmul(
                                pA[:], wmat[:, kt, 0:128], xhat[:, kt, nsl],
                                start=(kt == 0), stop=(kt == 7),
                            )
                        pB = psum.tile([128, QW], F32, tag="acc", bufs=3)
                        for kt in range(8):
                            nc.tensor.matmul(
                                pB[:], wmat[:, kt, 128:256], xhat[:, kt, nsl],
                                start=(kt == 0), stop=(kt == 7),
                            )
                        # RoPE: A' = A*cosA - B*sinA ; B' = B*cosB + A*sinB
                        t1 = ab.tile([128, QW], F32, tag="r1", bufs=2)
                        t2 = ab.tile([128, QW], F32, tag="r2", bufs=2)
                        nc.vector.tensor_tensor(t1[:], pA[:], cs["cosA"][:, nsl], ALU.mult)
                        nc.vector.tensor_tensor(t2[:], pB[:], cs["sinA"][:, nsl], ALU.mult)
                        nc.vector.tensor_tensor(outAB[0][:, nsl], t1[:], t2[:], ALU.subtract)
                        t3 = ab.tile([128, QW], F32, tag="r3", bufs=2)
                        t4 = ab.tile([128, QW], F32, tag="r4", bufs=2)
                        nc.vector.tensor_tensor(t3[:], pB[:], cs["cosB"][:, nsl], ALU.mult)
                        nc.vector.tensor_tensor(t4[:], pA[:], cs["sinB"][:, nsl], ALU.mult)
                        nc.vector.tensor_tensor(outAB[1][:, nsl], t3[:], t4[:], ALU.add)
                for ti in range(NT):
                    vps = psum.tile([128, 256], F32, tag="acc", bufs=3)
                    for kt in range(8):
                        nc.tensor.matmul(
                            vps[:], xhat[:, kt, ti * 128:(ti + 1) * 128], wv[:, kt, :],
                            start=(kt == 0), stop=(kt == 7),
                        )
                    for h in range(4):
                        nc.any.tensor_copy(
                            v_sb[:, ti, h, 0:64], vps[:, h * 64:(h + 1) * 64]
                        )

            # ---- Phases C+D per chunk ----
            cd_ctx = tc.tile_pool(name="cd", bufs=1)
            cd = cd_ctx.__enter__()
            for j in range(NCH):
                csl = slice(j * CT, (j + 1) * CT)
                # attention for this q-chunk
                av_t = cd.tile([128, 2, CT], BF16, tag="av_sb", bufs=2)
                for h in range(4):
                    g, r0 = h // 2, 64 * (h % 2)
                    rows = slice(r0, r0 + 64)
                    p_sb = cd.tile([128, NT, CT], BF16, tag="p", bufs=1)
                    avp = psum.tile([128, CT], F32, tag="av", bufs=1)
                    for kt in range(NT):
                        sc = psum.tile([128, CT], F32, tag="sc", bufs=2)
                        nc.tensor.matmul(
                            sc[:],
                            k_t[g][rows, kt * 128:(kt + 1) * 128],
                            q_t[g][rows, csl],
                            start=True, stop=True,
                        )
                        nc.scalar.activation(p_sb[:, kt, :], sc[:], AF.Exp)
                        nc.tensor.matmul(
                            avp[0:65, :], v_sb[:, kt, h, :], p_sb[:, kt, :],
                            start=(kt == 0), stop=(kt == NT - 1),
                        )
                    r_sb = cd.tile([1, CT], F32, tag="r_sb", bufs=2)
                    nc.vector.reciprocal(r_sb[:], avp[64:65, :])
                    bc = psum.tile([128, CT], F32, tag="tp", bufs=2)
                    nc.tensor.matmul(bc[0:64, :], ones64[:], r_sb[:],
                                     start=True, stop=True)
                    avn = cd.tile([64, CT], F32, tag="avn", bufs=2)
                    nc.scalar.copy(avn[:], avp[0:64, :])
                    nc.vector.tensor_tensor(
                        av_t[rows.start:rows.start + 64, g, :],
                        avn[:], bc[0:64, :], ALU.mult,
                    )
                # out-proj + src/4, AR1
                for m in range(CM):
                    tsl = slice(j * CT + m * 128, j * CT + (m + 1) * 128)
                    src4_c = stream.tile([128, D], F32, tag="stream", bufs=4, name="src4_c")
                    nc.sync.dma_start(src4_c[:], xsrc4[tsl, :])
                    o_sb = cd.tile([128, D], F32, tag="o_sb", bufs=2)
                    for n in range(2):
                        po = psum.tile([128, 512], F32, tag="acc", bufs=3)
                        for g in range(2):
                            nc.tensor.matmul(
                                po[:],
                                av_t[:, g, m * 128:(m + 1) * 128],
                                wo[:, g, n * 512:(n + 1) * 512],
                                start=(g == 0), stop=(g == 1),
                            )
                        nc.vector.tensor_tensor(
                            o_sb[:, n * 512:(n + 1) * 512], po[:],
                            src4_c[:, n * 512:(n + 1) * 512], ALU.add,
                        )
                    nc.sync.dma_start(ar1_in[tsl, :], o_sb[:])
            nc.gpsimd.collective_compute(
                "AllReduce", ALU.add, replica_groups=groups,
                ins=[ar1_in[:, :]], outs=[ar1_out[:, :]],
            )
            for j in range(NCH):
                csl = slice(j * CT, (j + 1) * CT)
                # ---- Phase D: LN2 + FFN ----
                xhat2 = cd.tile([128, 8, CT], BF16, tag="xhat2", bufs=2)
                src24 = []
                for m in range(CM):
                    tsl = slice(j * CT + m * 128, j * CT + (m + 1) * 128)
                    s2 = stream.tile([128, D], F32, tag="stream", bufs=4, name="s2")
                    nc.sync.dma_start(s2[:], ar1_out[tsl, :])
                    st = work.tile([128, 2, 6], F32, tag="st2")
                    nc.vector.bn_stats(st[:, 0, :], s2[:, 0:512])
                    nc.vector.bn_stats(st[:, 1, :], s2[:, 512:1024])
                    mv = work.tile([128, 2], F32, tag="mv2")
                    nc.vector.bn_aggr(mv[:], st[:])
                    vareps = work.tile([128, 1], F32, tag="ve2")
                    nc.vector.tensor_scalar_add(vareps[:], mv[:, 1:2], EPS)
                    stdv = work.tile([128, 1], F32, tag="sd2")
                    nc.scalar.activation(stdv[:], vareps[:], AF.Sqrt)
                    rstd = work.tile([128, 1], F32, tag="rs2")
                    nc.vector.reciprocal(rstd[:], stdv[:])
                    xn2 = work.tile([128, D], BF16, tag="xn2")
                    nc.vector.tensor_scalar(
                        xn2[:], s2[:], mv[:, 0:1], rstd[:], ALU.subtract, ALU.mult,
                    )
                    s24 = cd.tile([128, D], F32, tag="s24", bufs=4)
                    nc.scalar.activation(s24[:], s2[:], AF.Copy, scale=0.25)
                    src24.append(s24)
                    for half in range(2):
                        tp = psum.tile([128, 4, 128], BF16, tag="tp", bufs=2)
                        for c in range(4):
                            nc.tensor.transpose(
                                tp[:, c, :],
                                xn2[:, (half * 4 + c) * 128:(half * 4 + c + 1) * 128],
                                ident[:],
                            )
                        nc.any.tensor_copy(
                            xhat2[:, half * 4:half * 4 + 4, m * 128:(m + 1) * 128],
                            tp[:],
                        )
                h_sb = cd.tile([128, NFF, CT], BF16, tag="h_sb", bufs=1)
                for f in range(NFF):
                    gps = psum.tile([128, CT], F32, tag="acc", bufs=3)
                    ups = psum.tile([128, CT], F32, tag="acc", bufs=3)
                    for kt in range(8):
                        nc.tensor.matmul(
                            gps[:], w1[:, kt, f * 128:(f + 1) * 128], xhat2[:, kt, :],
                            start=(kt == 0), stop=(kt == 7),
                        )
                    for kt in range(8):
                        nc.tensor.matmul(
                            ups[:], w3[:, kt, f * 128:(f + 1) * 128], xhat2[:, kt, :],
                            start=(kt == 0), stop=(kt == 7),
                        )
                    if use_silu:
                        sil = cd.tile([128, CT], F32, tag="sil", bufs=2)
                        nc.scalar.activation(sil[:], gps[:], AF.Silu)
                        nc.vector.tensor_tensor(h_sb[:, f, :], sil[:], ups[:], ALU.mult)
                    else:
                        sig = cd.tile([128, CT], F32, tag="sil", bufs=2)
                        nc.scalar.activation(sig[:], gps[:], AF.Sigmoid)
                        gu = cd.tile([128, CT], F32, tag="gu", bufs=2)
                        nc.vector.tensor_tensor(gu[:], gps[:], ups[:], ALU.mult)
                        nc.vector.tensor_tensor(h_sb[:, f, :], gu[:], sig[:], ALU.mult)
                for m in range(CM):
                    tsl = slice(j * CT + m * 128, j * CT + (m + 1) * 128)
                    a2_sb = cd.tile([128, D], F32, tag="a2_sb", bufs=2)
                    for n in range(2):
                        dp = psum.tile([128, 512], F32, tag="acc", bufs=3)
                        for kt in range(NFF):
                            nc.tensor.matmul(
                                dp[:],
                                h_sb[:, kt, m * 128:(m + 1) * 128],
                                w2[:, kt, n * 512:(n + 1) * 512],
                                start=(kt == 0), stop=(kt == NFF - 1),
                            )
                        nc.vector.tensor_tensor(
                            a2_sb[:, n * 512:(n + 1) * 512], dp[:],
                            src24[m][:, n * 512:(n + 1) * 512], ALU.add,
                        )
                    nc.sync.dma_start(ar2_in[tsl, :], a2_sb[:])
            nc.gpsimd.collective_compute(
                "AllReduce", ALU.add, replica_groups=groups,
                ins=[ar2_in[:, :]], outs=[ar2_out[:, :]],
            )
            nc.sync.dma_start(y[:, :], ar2_out[:, :])
            cd_ctx.__exit__(None, None, None)

    if wait_split:
        split_all_waits(nc)
    return nc


# ---------------------------------------------------------------------------
# Host side
# ---------------------------------------------------------------------------
def make_in_maps(inputs, S=2048):
    src = np.asarray(inputs["src"], np.float32)
    cos = np.asarray(inputs["cos"], np.float32)
    sin = np.asarray(inputs["sin"], np.float32)
    g1 = np.asarray(inputs["g1"], np.float32)
    g2 = np.asarray(inputs["g2"], np.float32)
    for nm in ("bq", "bk", "bv", "bo", "b1", "b2"):
        assert not np.any(np.asarray(inputs[nm])), f"{nm} must be zero"
    assert not np.any(np.asarray(inputs["src_key_padding_mask"])), "mask must be False"
    Wq = np.asarray(inputs["Wq"], np.float32) * g1[None, :]
    Wk = np.asarray(inputs["Wk"], np.float32) * g1[None, :]
    Wv = np.asarray(inputs["Wv"], np.float32) * g1[None, :]
    Wo = np.asarray(inputs["Wo"], np.float32)
    W1 = np.asarray(inputs["W1"], np.float32) * g2[None, :]
    W3 = np.asarray(inputs["W3"], np.float32) * g2[None, :]
    W2 = np.asarray(inputs["W2"], np.float32)
    cosT, sinT = np.ascontiguousarray(cos.T), np.ascontiguousarray(sin.T)

    in_maps = []
    for c in range(N_CORES):
        b, jj = c // 4, c % 4
        A0 = 128 * jj
        chansA = np.arange(A0, A0 + 128)
        chansB = 512 + chansA
        chans = np.concatenate([chansA, chansB])
        ffsl = slice((FF // 4) * jj, (FF // 4) * (jj + 1))

        def bft(x):
            return np.ascontiguousarray(x).astype(BF)

        m = {
            "xsrc4": np.ascontiguousarray(src[b] * 0.25),
            "cosA": bft(cosT[chansA]), "sinA": bft(sinT[chansA]),
            "cosB": bft(cosT[chansB]), "sinB": bft(sinT[chansB]),
            "wq": bft((Wq[chans, :].T / 8.0).reshape(8, 128, 256).transpose(1, 0, 2)),
            "wk": bft(Wk[chans, :].T.reshape(8, 128, 256).transpose(1, 0, 2)),
            "wv": bft(Wv[chans, :].T.reshape(8, 128, 256).transpose(1, 0, 2)),
            "wo": bft(Wo[:, chans].T.reshape(2, 128, D).transpose(1, 0, 2)),
            "w1t": bft(W1[ffsl, :].T.reshape(8, 128, FF // 4).transpose(1, 0, 2)),
            "w3t": bft(W3[ffsl, :].T.reshape(8, 128, FF // 4).transpose(1, 0, 2)),
            "w2t": bft(W2[:, ffsl].T.reshape(8, 128, D).transpose(1, 0, 2)),
        }
        in_maps.append(m)
    return in_maps


_CACHE = {}


def kernel(**inputs) -> np.ndarray:
    S = np.asarray(inputs["src"]).shape[1]
    if S not in _CACHE:
        _CACHE[S] = build_bass(S=S)
    nc = _CACHE[S]
    in_maps = make_in_maps(inputs, S=S)
    res = run_bass_kernel_spmd(nc, in_maps, list(range(N_CORES)))
    out = np.stack([res.results[0]["y"], res.results[4]["y"]])
    return out.astype(np.float32)


if __name__ == "__main__":
    import reference

    inputs = reference.setup_inputs()
    expected = np.asarray(reference.reference(**inputs))
    actual = kernel(**{k: np.asarray(v) for k, v in inputs.items()})
    rel = np.linalg.norm(actual - expected) / np.linalg.norm(expected)
    print("Relative error:", rel)



# revision 16
# speedup vs baseline: 1.0696x; 1.0696x over previous
"""Trainium2 Bass kernel for a custom transformer encoder layer
(pre-LN, RoPE-on-full-d_model attention, SwiGLU FFN).

Sharding: DP over batch (2 groups of 4 cores) x TP within group:
  - attention: 4 heads/core (head pairs {h, h+8} so RoPE stays local),
  - FFN: dim_feedforward/4 per core,
  - residuals folded into the two grouped AllReduces (src/4 added on every
    core pre-AR so the CCE sum reconstructs src exactly).

Dataflow per core (batch b, core j in group):
  LN1 (token-major, stats on src/4 with eps/16 -> scale-invariant)
  -> PE-transpose to feature-major bf16 x_hat
  -> q,k feature-major + fused RoPE; v token-major (+ones col for softmax sums)
  -> per 512-token chunk: scores^T = k^T q per head -> exp -> av matmul with
     ones-row giving softmax sums -> reciprocal+K=1-broadcast-matmul normalize
  -> out-proj (token-major) + src/4 -> AllReduce(group) = src2
  -> LN2 token-major -> transpose -> SwiGLU FFN (ff-slice) -> down-proj
     (token-major) + src2/4 -> AllReduce(group) = final output chunk.
"""
import sys

sys.path.insert(0, "/opt/trn_rl_repo")

import numpy as np
import ml_dtypes

import concourse.bass as bass
import concourse.mybir as mybir
from concourse import masks, tile
from concourse.bass_utils import run_bass_kernel_spmd

BF = ml_dtypes.bfloat16
F32 = mybir.dt.float32
BF16 = mybir.dt.bfloat16
AF = mybir.ActivationFunctionType
ALU = mybir.AluOpType

B, D, H, HD, FF = 2, 1024, 16, 64, 4096
EPS = 1e-5
N_CORES = 8

# ---------------------------------------------------------------------------
# Workaround: this neuronxcc build rejects >1 sem wait on the CTRL-queue (SP)
# drain Tile emits at context exit. Split extra waits onto chained SP nops
# (same FIFO queue -> semantics preserved).
_MAXW = 1


def _split_waits(nc, inst):
    si = inst.sync_info
    if si is None:
        return
    waits = list(si.on_wait)
    if len(waits) <= _MAXW:
        return
    inst.sync_info = mybir.SyncInfo(on_wait=waits[:_MAXW], on_update=list(si.on_update))
    for i in range(_MAXW, len(waits), _MAXW):
        ni = nc.sync.nop(nofuse=True)
        ni.ins.sync_info = mybir.SyncInfo(on_wait=waits[i : i + _MAXW], on_update=[])


_SPLIT_N = [0]


def split_all_waits(nc, maxw=1):
    """Post-pass: this walrus build rejects instructions carrying more than a
    couple of sem waits. Move extra waits onto same-engine nops inserted
    immediately before the offending instruction (per-engine FIFO order
    makes this equivalent)."""
    for f in nc.m.functions:
        for bb in f.blocks:
            out = []
            changed = False
            for inst in bb.instructions:
                si = getattr(inst, "sync_info", None)
                waits = list(si.on_wait) if si is not None else []
                if len(waits) > maxw:
                    for i in range(maxw, len(waits), maxw):
                        _SPLIT_N[0] += 1
                        nop = mybir.InstNoOp(
                            name=f"I-wsplit-{_SPLIT_N[0]}", engine=inst.engine,
                            ins=[], outs=[],
                        )
                        nop.sync_info = mybir.SyncInfo(
                            on_wait=waits[i:i + maxw], on_update=[]
                        )
                        out.append(nop)
                        changed = True
                    inst.sync_info = mybir.SyncInfo(
                        on_wait=waits[:maxw], on_update=list(si.on_update)
                    )
                out.append(inst)
            if changed:
                bb.instructions = out


def _patched_drain_and_barrier(self, tick_clock, wait_clock):
    nc = self.nc
    drain_inst = nc.sync.drain()
    wait_clock.add_sem_waits(
        drain_inst.ins, tile.ScopedClock({None: tick_clock.global_clock})
    )
    _split_waits(nc, drain_inst.ins)
    nc.all_engine_barrier()
    assert self.sems is not None
    popped = nc._tile_sem_poison_stack.pop()
    assert popped is self._sem_poison
    nc.clear_and_free_semaphores(list(self.sems.allocated().values()))
    nc.all_engine_barrier()


tile.TileContext._drain_and_barrier = _patched_drain_and_barrier
# ---------------------------------------------------------------------------


def build_bass(S=2048, CT=512, use_silu=True, wait_split=True):
    """Emit the SPMD program. CT = tokens per pipeline chunk."""
    NT = S // 128          # 128-token tiles
    NCH = S // CT          # chunks
    CM = CT // 128         # 128-token tiles per chunk
    NQ = CT // 512 if CT >= 512 else 1  # q-tile width for qk projections
    QW = min(S, 512)       # matmul N for q/k projection over full seq
    NQK = S // QW

    nc = bass.Bass(target_bir_lowering=False, debug=False)

    # --- I/O ---
    # All bf16 params packed into ONE dram tensor (per-call host dispatch
    # overhead scales with arg count): flat [128, NCOL] column layout.
    COSA, SINA, COSB, SINB = 0, S, 2 * S, 3 * S
    WQ = 4 * S
    WK = WQ + 2048
    WV = WK + 2048
    WO = WV + 2048
    W1 = WO + 2048
    W3 = W1 + 8192
    W2 = W3 + 8192
    NCOL = W2 + 8192
    xsrc4 = nc.dram_tensor("xsrc4", [S, D], F32, kind="ExternalInput")
    wpack = nc.dram_tensor("wpack", [128, NCOL], BF16, kind="ExternalInput")
    y = nc.dram_tensor("y", [S // 4, D], F32, kind="ExternalOutput")

    # --- internal DRAM for collectives ---
    ar1_in = nc.dram_tensor("ar1_in", [S, D], F32)
    ar1_out = nc.dram_tensor("ar1_out", [S, D], F32)
    ar2_in = nc.dram_tensor("ar2_in", [S, D], F32)
    ar2_out = nc.dram_tensor("ar2_out", [S // 4, D], F32)
    groups = [[0, 1, 2, 3], [4, 5, 6, 7]]

    FFS = FF // 4          # ff slice per core
    NFF = FFS // 128

    with tile.TileContext(nc) as tc:
        with (
            tc.tile_pool(name="consts", bufs=1) as cpool,
            tc.tile_pool(name="weights", bufs=1) as wpool,
            tc.tile_pool(name="persist", bufs=1) as ppool,
            tc.tile_pool(name="psum", bufs=1, space="PSUM") as psum,
            tc.tile_pool(name="work", bufs=2) as work,
            tc.tile_pool(name="stream", bufs=3) as stream,
        ):
            # consts
            ident = cpool.tile([128, 128], BF16)
            masks.make_identity(nc, ident[:])
            ones64 = cpool.tile([1, 64], F32)
            nc.vector.memset(ones64[:], 1.0)

            # packed weights -> SBUF in one DMA (cos/sin stay in the
            # transient ab pool below; persistent SBUF is tight)
            wsb_ = wpool.tile([128, NCOL - WQ], BF16)
            nc.sync.dma_start(wsb_[:], wpack[:, WQ:NCOL])

            class _Shifted:
                def __getitem__(self, idx):
                    rows, cols = idx
                    return wsb_[rows, cols.start - WQ:cols.stop - WQ]

            wsb = _Shifted()

            # persistent activations
            q_t = [ppool.tile([128, S], BF16, tag=f"q{g}", name=f"q{g}")
                   for g in range(2)]
            k_t = [ppool.tile([128, S], BF16, tag=f"k{g}", name=f"k{g}")
                   for g in range(2)]
            v_sb = ppool.tile([128, NT, 4, 65], BF16)
            nc.vector.memset(v_sb[:, :, :, 64:65], 1.0)

            with (
                tc.tile_pool(name="ab", bufs=1) as ab,
            ):
                xhat = ab.tile([128, 8, S], BF16)
                cs = {}
                for nm, c0 in (("cosA", COSA), ("sinA", SINA),
                               ("cosB", COSB), ("sinB", SINB)):
                    t = ab.tile([128, S], BF16, tag=nm)
                    nc.sync.dma_start(t[:], wpack[:, c0:c0 + S])
                    cs[nm] = t

                # ---- Phase A: LN1 + transpose ----
                for ti in range(NT):
                    sl = slice(ti * 128, ti * 128 + 128)
                    src_t = stream.tile([128, D], F32, tag="stream", bufs=4)
                    nc.sync.dma_start(src_t[:], xsrc4[sl, :])
                    st = work.tile([128, 2, 6], F32, tag="st")
                    nc.vector.bn_stats(st[:, 0, :], src_t[:, 0:512])
                    nc.vector.bn_stats(st[:, 1, :], src_t[:, 512:1024])
                    mv = work.tile([128, 2], F32, tag="mv")
                    nc.vector.bn_aggr(mv[:], st[:])
                    vareps = work.tile([128, 1], F32, tag="ve")
                    nc.vector.tensor_scalar_add(vareps[:], mv[:, 1:2], EPS / 16.0)
                    stdv = work.tile([128, 1], F32, tag="sd")
                    nc.scalar.activation(stdv[:], vareps[:], AF.Sqrt)
                    rstd = work.tile([128, 1], F32, tag="rs")
                    nc.vector.reciprocal(rstd[:], stdv[:])
                    xn = work.tile([128, D], BF16, tag="xn")
                    nc.vector.tensor_scalar(
                        xn[:], src_t[:], mv[:, 0:1], rstd[:],
                        ALU.subtract, ALU.mult,
                    )
                    for half in range(2):
                        tp = psum.tile([128, 4, 128], BF16, tag="tp", bufs=2)
                        for c in range(4):
                            nc.tensor.transpose(
                                tp[:, c, :],
                                xn[:, (half * 4 + c) * 128:(half * 4 + c + 1) * 128],
                                ident[:],
                            )
                        nc.any.tensor_copy(xhat[:, half * 4:half * 4 + 4, sl], tp[:])

                # ---- Phase B: q, k (+RoPE) and v ----
                for which, W0, outAB in (("k", WK, k_t), ("q", WQ, q_t)):
                    for ntl in range(NQK):
                        nsl = slice(ntl * QW, (ntl + 1) * QW)
                        pA = psum.tile([128, QW], F32, tag="acc", bufs=3)
                        for kt in range(8):
                            nc.tensor.matmul(
                                pA[:],
                                wsb[:, W0 + kt * 256:W0 + kt * 256 + 128],
                                xhat[:, kt, nsl],
                                start=(kt == 0), stop=(kt == 7),
                            )
                        pB = psum.tile([128, QW], F32, tag="acc", bufs=3)
                        for kt in range(8):
                            nc.tensor.matmul(
                                pB[:],
                                wsb[:, W0 + kt * 256 + 128:W0 + kt * 256 + 256],
                                xhat[:, kt, nsl],
                                start=(kt == 0), stop=(kt == 7),
                            )
                        # RoPE: A' = A*cosA - B*sinA ; B' = B*cosB + A*sinB
                        t1 = ab.tile([128, QW], F32, tag="r1", bufs=2)
                        t2 = ab.tile([128, QW], F32, tag="r2", bufs=2)
                        nc.vector.tensor_tensor(t1[:], pA[:], cs["cosA"][:, nsl], ALU.mult)
                        nc.vector.tensor_tensor(t2[:], pB[:], cs["sinA"][:, nsl], ALU.mult)
                        nc.vector.tensor_tensor(outAB[0][:, nsl], t1[:], t2[:], ALU.subtract)
                        t3 = ab.tile([128, QW], F32, tag="r3", bufs=2)
                        t4 = ab.tile([128, QW], F32, tag="r4", bufs=2)
                        nc.vector.tensor_tensor(t3[:], pB[:], cs["cosB"][:, nsl], ALU.mult)
                        nc.vector.tensor_tensor(t4[:], pA[:], cs["sinB"][:, nsl], ALU.mult)
                        nc.vector.tensor_tensor(outAB[1][:, nsl], t3[:], t4[:], ALU.add)
                for ti in range(NT):
                    vps = psum.tile([128, 256], F32, tag="acc", bufs=3)
                    for kt in range(8):
                        nc.tensor.matmul(
                            vps[:], xhat[:, kt, ti * 128:(ti + 1) * 128],
                            wsb[:, WV + kt * 256:WV + (kt + 1) * 256],
                            start=(kt == 0), stop=(kt == 7),
                        )
                    for h in range(4):
                        nc.any.tensor_copy(
                            v_sb[:, ti, h, 0:64], vps[:, h * 64:(h + 1) * 64]
                        )

            # ---- Phases C+D per chunk ----
            cd_ctx = tc.tile_pool(name="cd", bufs=1)
            cd = cd_ctx.__enter__()
            for j in range(NCH):
                csl = slice(j * CT, (j + 1) * CT)
                # attention for this q-chunk
                av_t = cd.tile([128, 2, CT], BF16, tag="av_sb", bufs=2)
                for h in range(4):
                    g, r0 = h // 2, 64 * (h % 2)
                    rows = slice(r0, r0 + 64)
                    p_sb = cd.tile([128, NT, CT], BF16, tag="p", bufs=1)
                    avp = psum.tile([128, CT], F32, tag="av", bufs=1)
                    for kt in range(NT):
                        sc = psum.tile([128, CT], F32, tag="sc", bufs=2)
                        nc.tensor.matmul(
                            sc[:],
                            k_t[g][rows, kt * 128:(kt + 1) * 128],
                            q_t[g][rows, csl],
                            start=True, stop=True,
                        )
                        nc.scalar.activation(p_sb[:, kt, :], sc[:], AF.Exp)
                        nc.tensor.matmul(
                            avp[0:65, :], v_sb[:, kt, h, :], p_sb[:, kt, :],
                            start=(kt == 0), stop=(kt == NT - 1),
                        )
                    r_sb = cd.tile([1, CT], F32, tag="r_sb", bufs=2)
                    nc.vector.reciprocal(r_sb[:], avp[64:65, :])
                    bc = psum.tile([128, CT], F32, tag="tp", bufs=2)
                    nc.tensor.matmul(bc[0:64, :], ones64[:], r_sb[:],
                                     start=True, stop=True)
                    avn = cd.tile([64, CT], F32, tag="avn", bufs=2)
                    nc.scalar.copy(avn[:], avp[0:64, :])
                    nc.vector.tensor_tensor(
                        av_t[rows.start:rows.start + 64, g, :],
                        avn[:], bc[0:64, :], ALU.mult,
                    )
                # out-proj + src/4, AR1
                for m in range(CM):
                    tsl = slice(j * CT + m * 128, j * CT + (m + 1) * 128)
                    src4_c = stream.tile([128, D], F32, tag="stream", bufs=4, name="src4_c")
                    nc.sync.dma_start(src4_c[:], xsrc4[tsl, :])
                    o_sb = cd.tile([128, D], F32, tag="o_sb", bufs=2)
                    for n in range(2):
                        po = psum.tile([128, 512], F32, tag="acc", bufs=3)
                        for g in range(2):
                            nc.tensor.matmul(
                                po[:],
                                av_t[:, g, m * 128:(m + 1) * 128],
                                wsb[:, WO + g * 1024 + n * 512:
                                     WO + g * 1024 + (n + 1) * 512],
                                start=(g == 0), stop=(g == 1),
                            )
                        nc.vector.tensor_tensor(
                            o_sb[:, n * 512:(n + 1) * 512], po[:],
                            src4_c[:, n * 512:(n + 1) * 512], ALU.add,
                        )
                    nc.sync.dma_start(ar1_in[tsl, :], o_sb[:])
            nc.gpsimd.collective_compute(
                "AllReduce", ALU.add, replica_groups=groups,
                ins=[ar1_in[:, :]], outs=[ar1_out[:, :]],
            )
            for j in range(NCH):
                csl = slice(j * CT, (j + 1) * CT)
                # ---- Phase D: LN2 + FFN ----
                xhat2 = cd.tile([128, 8, CT], BF16, tag="xhat2", bufs=2)
                src24 = []
                for m in range(CM):
                    tsl = slice(j * CT + m * 128, j * CT + (m + 1) * 128)
                    s2 = stream.tile([128, D], F32, tag="stream", bufs=4, name="s2")
                    nc.sync.dma_start(s2[:], ar1_out[tsl, :])
                    st = work.tile([128, 2, 6], F32, tag="st2")
                    nc.vector.bn_stats(st[:, 0, :], s2[:, 0:512])
                    nc.vector.bn_stats(st[:, 1, :], s2[:, 512:1024])
                    mv = work.tile([128, 2], F32, tag="mv2")
                    nc.vector.bn_aggr(mv[:], st[:])
                    vareps = work.tile([128, 1], F32, tag="ve2")
                    nc.vector.tensor_scalar_add(vareps[:], mv[:, 1:2], EPS)
                    stdv = work.tile([128, 1], F32, tag="sd2")
                    nc.scalar.activation(stdv[:], vareps[:], AF.Sqrt)
                    rstd = work.tile([128, 1], F32, tag="rs2")
                    nc.vector.reciprocal(rstd[:], stdv[:])
                    xn2 = work.tile([128, D], BF16, tag="xn2")
                    nc.vector.tensor_scalar(
                        xn2[:], s2[:], mv[:, 0:1], rstd[:], ALU.subtract, ALU.mult,
                    )
                    s24 = cd.tile([128, D], F32, tag="s24", bufs=4)
                    nc.scalar.activation(s24[:], s2[:], AF.Copy, scale=0.25)
                    src24.append(s24)
                    for half in range(2):
                        tp = psum.tile([128, 4, 128], BF16, tag="tp", bufs=2)
                        for c in range(4):
                            nc.tensor.transpose(
                                tp[:, c, :],
                                xn2[:, (half * 4 + c) * 128:(half * 4 + c + 1) * 128],
                                ident[:],
                            )
                        nc.any.tensor_copy(
                            xhat2[:, half * 4:half * 4 + 4, m * 128:(m + 1) * 128],
                            tp[:],
                        )
                h_sb = cd.tile([128, NFF, CT], BF16, tag="h_sb", bufs=1)
                for f in range(NFF):
                    gps = psum.tile([128, CT], F32, tag="acc", bufs=3)
                    ups = psum.tile([128, CT], F32, tag="acc", bufs=3)
                    for kt in range(8):
                        nc.tensor.matmul(
                            gps[:],
                            wsb[:, W1 + kt * 1024 + f * 128:
                                 W1 + kt * 1024 + (f + 1) * 128],
                            xhat2[:, kt, :],
                            start=(kt == 0), stop=(kt == 7),
                        )
                    for kt in range(8):
                        nc.tensor.matmul(
                            ups[:],
                            wsb[:, W3 + kt * 1024 + f * 128:
                                 W3 + kt * 1024 + (f + 1) * 128],
                            xhat2[:, kt, :],
                            start=(kt == 0), stop=(kt == 7),
                        )
                    if use_silu:
                        sil = cd.tile([128, CT], F32, tag="sil", bufs=2)
                        nc.scalar.activation(sil[:], gps[:], AF.Silu)
                        nc.vector.tensor_tensor(h_sb[:, f, :], sil[:], ups[:], ALU.mult)
                    else:
                        sig = cd.tile([128, CT], F32, tag="sil", bufs=2)
                        nc.scalar.activation(sig[:], gps[:], AF.Sigmoid)
                        gu = cd.tile([128, CT], F32, tag="gu", bufs=2)
                        nc.vector.tensor_tensor(gu[:], gps[:], ups[:], ALU.mult)
                        nc.vector.tensor_tensor(h_sb[:, f, :], gu[:], sig[:], ALU.mult)
                for m in range(CM):
                    tsl = slice(j * CT + m * 128, j * CT + (m + 1) * 128)
                    a2_sb = cd.tile([128, D], F32, tag="a2_sb", bufs=2)
                    for n in range(2):
                        dp = psum.tile([128, 512], F32, tag="acc", bufs=3)
                        for kt in range(NFF):
                            nc.tensor.matmul(
                                dp[:],
                                h_sb[:, kt, m * 128:(m + 1) * 128],
                                wsb[:, W2 + kt * 1024 + n * 512:
                                     W2 + kt * 1024 + (n + 1) * 512],
                                start=(kt == 0), stop=(kt == NFF - 1),
                            )
                        nc.vector.tensor_tensor(
                            a2_sb[:, n * 512:(n + 1) * 512], dp[:],
                            src24[m][:, n * 512:(n + 1) * 512], ALU.add,
                        )
                    nc.sync.dma_start(ar2_in[tsl, :], a2_sb[:])
            # Final collective as ReduceScatter: each core receives only its
            # own S/4 token shard of the summed output -> 4x smaller y.
            nc.gpsimd.collective_compute(
                "ReduceScatter", ALU.add, replica_groups=groups,
                ins=[ar2_in[:, :]], outs=[ar2_out[:, :]],
            )
            nc.sync.dma_start(y[:, :], ar2_out[:, :])
            cd_ctx.__exit__(None, None, None)

    if wait_split:
        split_all_waits(nc)
    return nc


# ---------------------------------------------------------------------------
# Host side
# ---------------------------------------------------------------------------
def make_in_maps(inputs, S=2048):
    src = np.asarray(inputs["src"], np.float32)
    cos = np.asarray(inputs["cos"], np.float32)
    sin = np.asarray(inputs["sin"], np.float32)
    g1 = np.asarray(inputs["g1"], np.float32)
    g2 = np.asarray(inputs["g2"], np.float32)
    for nm in ("bq", "bk", "bv", "bo", "b1", "b2"):
        assert not np.any(np.asarray(inputs[nm])), f"{nm} must be zero"
    assert not np.any(np.asarray(inputs["src_key_padding_mask"])), "mask must be False"
    Wq = np.asarray(inputs["Wq"], np.float32) * g1[None, :]
    Wk = np.asarray(inputs["Wk"], np.float32) * g1[None, :]
    Wv = np.asarray(inputs["Wv"], np.float32) * g1[None, :]
    Wo = np.asarray(inputs["Wo"], np.float32)
    W1 = np.asarray(inputs["W1"], np.float32) * g2[None, :]
    W3 = np.asarray(inputs["W3"], np.float32) * g2[None, :]
    W2 = np.asarray(inputs["W2"], np.float32)
    cosT, sinT = np.ascontiguousarray(cos.T), np.ascontiguousarray(sin.T)

    in_maps = []
    for c in range(N_CORES):
        b, jj = c // 4, c % 4
        A0 = 128 * jj
        chansA = np.arange(A0, A0 + 128)
        chansB = 512 + chansA
        chans = np.concatenate([chansA, chansB])
        ffsl = slice((FF // 4) * jj, (FF // 4) * (jj + 1))

        def bft(x):
            return np.ascontiguousarray(x).astype(BF)

        def flat(x):  # [128, a, b] -> [128, a*b]
            return x.reshape(128, -1)

        wpack = np.concatenate([
            bft(cosT[chansA]), bft(sinT[chansA]),
            bft(cosT[chansB]), bft(sinT[chansB]),
            flat(bft((Wq[chans, :].T / 8.0).reshape(8, 128, 256).transpose(1, 0, 2))),
            flat(bft(Wk[chans, :].T.reshape(8, 128, 256).transpose(1, 0, 2))),
            flat(bft(Wv[chans, :].T.reshape(8, 128, 256).transpose(1, 0, 2))),
            flat(bft(Wo[:, chans].T.reshape(2, 128, D).transpose(1, 0, 2))),
            flat(bft(W1[ffsl, :].T.reshape(8, 128, FF // 4).transpose(1, 0, 2))),
            flat(bft(W3[ffsl, :].T.reshape(8, 128, FF // 4).transpose(1, 0, 2))),
            flat(bft(W2[:, ffsl].T.reshape(8, 128, D).transpose(1, 0, 2))),
        ], axis=1)
        m = {
            "xsrc4": np.ascontiguousarray(src[b] * 0.25),
            "wpack": np.ascontiguousarray(wpack),
        }
        in_maps.append(m)
    return in_maps


def assemble_output(ycat: np.ndarray) -> np.ndarray:
    """[N_CORES*(S/4), D] concat of per-core ReduceScatter shards -> [B, S, D]."""
    y = ycat.reshape(N_CORES, -1, D)
    return np.stack([
        np.concatenate([y[0], y[1], y[2], y[3]], axis=0),
        np.concatenate([y[4], y[5], y[6], y[7]], axis=0),
    ]).astype(np.float32)


_CACHE = {}


def kernel(**inputs) -> np.ndarray:
    S = np.asarray(inputs["src"]).shape[1]
    if S not in _CACHE:
        _CACHE[S] = build_bass(S=S)
    nc = _CACHE[S]
    in_maps = make_in_maps(inputs, S=S)
    res = run_bass_kernel_spmd(nc, in_maps, list(range(N_CORES)))
    ycat = np.concatenate([res.results[c]["y"] for c in range(N_CORES)], axis=0)
    return assemble_output(ycat)


if __name__ == "__main__":
    import reference

    inputs = reference.setup_inputs()
    expected = np.asarray(reference.reference(**inputs))
    actual = kernel(**{k: np.asarray(v) for k, v in inputs.items()})
    rel = np.linalg.norm(actual - expected) / np.linalg.norm(expected)
    print("Relative error:", rel)

